# revision 1
# baseline (speedup 1.0000x reference)
"""Trainium2 Bass kernel for BPT attention wrapper with alibi (head-axis attention).

Sharding: 8 cores = 2 batches x 4 sequence-quarters (512 positions each).
Cross-core: one AllReduce of per-head Gram matrices G_h = Q_h^T Q_h within each
4-core batch group (pinv(Q) = G^{-1} Q^T needs full-sequence G).

Math per (b,s) position, per head pair (i,j):
  scores[i,j] = (q_i . k_j + sqrt(D) * alibi[j,s] * (q_i . z_j)) / D
  z_j = G_j^{-1} q_j     (Newton-Schulz inverse on device)
  attn = softmax_j(scores);  ctx_i = sum_j attn[i,j] v_j
  out = ctx @ dense_w.T + dense_b + residual
"""
import numpy as np
import ml_dtypes

import concourse.bass as bass
import concourse.mybir as mybir
from concourse import bacc, bass_isa
from concourse.tile import TileContext
from concourse.bass_utils import run_bass_kernel_spmd
from concourse.masks import make_identity

F32 = mybir.dt.float32
F32R = mybir.dt.float32r
BF16 = mybir.dt.bfloat16
AL = mybir.AluOpType
ACTF = mybir.ActivationFunctionType

B, S, H, D = 2, 2048, 16, 128
HID = H * D
N_CORES = 8
POS = 512                 # positions per core
NEWTON_ITERS = 7


def build_bass(pos=POS, use_cc=True, newton_iters=NEWTON_ITERS, groups=None, reps=1, phases='aznbc', wdt=F32R):
    """Build the per-core Bass program. pos must be a multiple of 128."""
    nblk = pos // 8
    nchunk = HID // 128           # 16 contraction chunks
    n_rw = 3 * HID // 512         # 12 row windows of 512 qkv rows

    nc = bacc.Bacc()

    hsT3 = nc.dram_tensor("hsT3", (128, nchunk * pos), wdt, kind="ExternalInput")
    qkvw = nc.dram_tensor("qkvw", (nchunk * n_rw, 128, 512), wdt, kind="ExternalInput")
    densew = nc.dram_tensor("densew", (nchunk * 4, 128, 512), wdt, kind="ExternalInput")
    resT = nc.dram_tensor("resT", (16, 128, pos), F32, kind="ExternalInput")
    albc = nc.dram_tensor("albc", (128, nblk * 128), BF16, kind="ExternalInput")
    mask = nc.dram_tensor("maskbd", (128, 128), BF16, kind="ExternalInput")
    qkvb48 = nc.dram_tensor("qkvb48", (128, 48), F32, kind="ExternalInput")
    denseb16 = nc.dram_tensor("denseb16", (128, 16), F32, kind="ExternalInput")
    outT = nc.dram_tensor("outT", (16, 128, pos), F32, kind="ExternalOutput")

    with TileContext(nc) as tc:
      for _rep in range(reps):
            with (
                tc.tile_pool(name="per", bufs=1) as per,
                tc.tile_pool(name="dram", bufs=1, space="DRAM") as dram,
            ):
                # persistent SBUF tensors
                t_qti = per.tile([128, nblk * 128], BF16, tag="qti")
                t_kti = per.tile([128, nblk * 128], BF16, tag="kti")
                t_vti = per.tile([128, nblk * 128], BF16, tag="vti")
                t_zti = per.tile([128, nblk * 128], BF16, tag="zti")
                t_mask = per.tile([128, 128], BF16, tag="mask")
                t_dnb = per.tile([128, 16], F32, tag="dnb")
                t_id16 = per.tile([128, 128], BF16, tag="id16")
                t_idf = per.tile([128, 128], F32, tag="idf")

                nc.sync.dma_start(t_mask[:], mask[:])
                nc.sync.dma_start(t_dnb[:], denseb16[:])
                make_identity(nc, t_idf[:])
                nc.vector.tensor_copy(t_id16[:], t_idf[:])

                def ikv_slices(t, h):
                    # [d, (blk, j, p)] -> strided per-head view [128, nblk, 8]
                    return t[:].rearrange("d (k j p) -> d k j p", k=nblk, j=H)[:, :, h, :]

                # ---- pool spanning phases A + A2 (pinv working set) ----
                spa_cm = tc.tile_pool(name="spa", bufs=1)
                spa = spa_cm.__enter__()
                t_qtp = spa.tile([128, H * pos], BF16, tag="qtp")
                t_albc = spa.tile([128, nblk * 128], BF16, tag="albc")
                t_g32 = spa.tile([128, H * 128], F32, tag="g32")
                t_g16 = spa.tile([128, H * 128], BF16, tag="g16")
                t_w = spa.tile([128, H * 128], BF16, tag="wall")
                t_diag = spa.tile([128, H], F32, tag="diag")
                t_cbc = spa.tile([128, H], F32, tag="cbc")
                nc.sync.dma_start(t_albc[:], albc[:])

                # ---------------- Phase A: QKV projection (+ per-head G) ----------------
                with (
                    tc.tile_pool(name="a_hs", bufs=1) as a_hs,
                    tc.tile_pool(name="a_w", bufs=18) as a_w,
                    tc.tile_pool(name="a_ps", bufs=1, space="PSUM") as a_ps,
                    tc.tile_pool(name="g_ps", bufs=2, space="PSUM") as g_ps,
                    tc.tile_pool(name="g_sb", bufs=3) as g_sb,
                ):
                    t_hsT = a_hs.tile([128, nchunk * pos], wdt, tag="hsT")
                    nc.sync.dma_start(t_hsT[:], hsT3[:])
                    t_qkvb = a_hs.tile([128, 48], F32, tag="qkvb")
                    nc.sync.dma_start(t_qkvb[:], qkvb48[:])
                    def emit_g_head(h):
                        gp = g_ps.tile([128, 128], F32, tag="gps")
                        for ccc in range(pos // 128):
                            qp_ps = g_ps.tile([128, 128], F32, tag="qpos_ps")
                            nc.tensor.matmul(
                                qp_ps[:],
                                t_qtp[:, h * pos + ccc * 128: h * pos + ccc * 128 + 128],
                                t_id16[:], start=True, stop=True)
                            qch = g_sb.tile([128, 128], BF16, tag="qch")
                            nc.scalar.activation(qch[:], qp_ps[:], ACTF.Copy)
                            nc.tensor.matmul(gp[:], qch[:], qch[:],
                                             start=(ccc == 0), stop=(ccc == pos // 128 - 1))
                        nc.vector.tensor_copy(t_g32[:, h * 128:(h + 1) * 128], gp[:])

                    for rw in range(n_rw):
                        psums = [a_ps.tile([128, pos], F32, tag=f"aps{rt}", name=f"aps{rw}_{rt}") for rt in range(4)]
                        for half in range(2):
                            wt = []
                            for cc in range(half * 8, half * 8 + 8):
                                w = a_w.tile([128, 512], wdt, tag="aw")
                                nc.sync.dma_start(w[:], qkvw[cc * n_rw + rw])
                                wt.append(w)
                            for rt in range(4):
                                for ci, cc in enumerate(range(half * 8, half * 8 + 8)):
                                    nc.tensor.matmul(
                                        psums[rt][:],
                                        wt[ci][:, rt * 128:(rt + 1) * 128],
                                        t_hsT[:, cc * pos:(cc + 1) * pos],
                                        start=(half == 0 and ci == 0),
                                        stop=(half == 1 and ci == 7))
                        for rt in range(4):
                            m = rw * 4 + rt
                            h, t = divmod(m, 3)
                            psv = psums[rt][:].rearrange("d (k p) -> d k p", p=8)
                            bias = t_qkvb[:, m:m + 1]
                            if t == 0:
                                nc.vector.tensor_scalar_add(ikv_slices(t_qti, h), psv, bias)
                                nc.scalar.activation(t_qtp[:, h * pos:(h + 1) * pos],
                                                     psums[rt][:], ACTF.Identity, bias=bias)
                                emit_g_head(h)
                            elif t == 1:
                                nc.vector.tensor_scalar_add(ikv_slices(t_kti, h), psv, bias)
                            else:
                                nc.vector.tensor_scalar_add(ikv_slices(t_vti, h), psv, bias)

                # ---------------- Phase A2: AllReduce G + Newton + z ----------------
                if use_cc:
                    ccin = dram.tile([128, H * 128], F32, tag="ccin")
                    ccout = dram.tile([128, H * 128], F32, tag="ccout")
                    nc.sync.dma_start(ccin[:], t_g32[:])
                    nc.gpsimd.collective_compute(
                        "AllReduce", AL.add,
                        replica_groups=groups or [[0, 1, 2, 3], [4, 5, 6, 7]],
                        ins=[ccin[:]], outs=[ccout[:]])
                    nc.sync.dma_start(t_g32[:], ccout[:])
                nc.vector.tensor_copy(t_g16[:], t_g32[:])

                with (
                    tc.tile_pool(name="n_sb", bufs=4) as n_sb,
                      tc.tile_pool(name="n_x", bufs=2) as n_x,
                      tc.tile_pool(name="n_ps", bufs=2, space="PSUM") as n_ps,
                  ):
                      # safe init: c_h = 1 / ||G_h||_inf  (max abs row sum >= lambda_max)
                      for h in range(H):
                          nc.vector.tensor_reduce(
                              t_diag[:, h:h + 1], t_g32[:, h * 128:(h + 1) * 128],
                              axis=mybir.AxisListType.X, op=AL.add,
                              apply_absolute_value=True)
                      nc.gpsimd.partition_all_reduce(t_cbc[:], t_diag[:], channels=128,
                                                     reduce_op=bass_isa.ReduceOp.max)
                      nc.vector.reciprocal(t_cbc[:], t_cbc[:])

                      xcur = {}
                      for h in range(H):
                          x0 = n_x.tile([128, 128], BF16, tag=f"x{h}")
                          nc.vector.tensor_scalar_mul(x0[:], t_id16[:], t_cbc[:, h:h + 1])
                          xcur[h] = x0
                      for it in range(newton_iters):
                          for h in range(H):
                              gsl = t_g16[:, h * 128:(h + 1) * 128]
                              yp = n_ps.tile([128, 128], F32, tag="yps")
                              nc.tensor.matmul(yp[:], gsl, xcur[h][:], start=True, stop=True)
                              ysb = n_sb.tile([128, 128], BF16, tag="ysb")
                              nc.scalar.activation(ysb[:], yp[:], ACTF.Copy)
                              zp = n_ps.tile([128, 128], F32, tag="zps")
                              nc.tensor.matmul(zp[:], xcur[h][:], ysb[:], start=True, stop=True)
                              if it == newton_iters - 1:
                                  xn_ap = t_w[:, h * 128:(h + 1) * 128]
                              else:
                                  xn = n_x.tile([128, 128], BF16, tag=f"x{h}")
                                  xn_ap = xn[:]
                              nc.vector.scalar_tensor_tensor(
                                  xn_ap, xcur[h][:], 2.0, zp[:], op0=AL.mult, op1=AL.subtract)
                              if it != newton_iters - 1:
                                  xcur[h] = xn

                      # z_h = W_h @ q_h, prescaled by sqrt(D)*alibi into ZTi
                      for h in range(H):
                          zp = n_ps.tile([128, pos], F32, tag="ztps")
                          nc.tensor.matmul(zp[:], t_w[:, h * 128:(h + 1) * 128],
                                           t_qtp[:, h * pos:(h + 1) * pos],
                                           start=True, stop=True)
                          nc.vector.tensor_tensor(
                              ikv_slices(t_zti, h),
                              zp[:].rearrange("d (k p) -> d k p", p=8),
                              ikv_slices(t_albc, h), op=AL.mult)

                spa_cm.__exit__(None, None, None)

                # ---------------- Phase B/C span: ctxT ----------------
                spb_cm = tc.tile_pool(name="spb", bufs=1)
                spb = spb_cm.__enter__()
                t_ctxT = spb.tile([128, H * pos], wdt, tag="ctxT")

                # ---------------- Phase B: block attention ----------------
                with (
                    tc.tile_pool(name="b_sb", bufs=3) as b_sb,
                    tc.tile_pool(name="b_ps", bufs=2, space="PSUM") as b_ps,
                ):
                    for blk in range(nblk):
                        sl = slice(blk * 128, (blk + 1) * 128)
                        sp = b_ps.tile([128, 128], F32, tag="sps")
                        nc.tensor.matmul(sp[:], t_kti[:, sl], t_qti[:, sl],
                                         start=True, stop=False)
                        nc.tensor.matmul(sp[:], t_zti[:, sl], t_qti[:, sl],
                                         start=False, stop=True)
                        esb = b_sb.tile([128, 128], BF16, tag="esb")
                        nc.scalar.activation(esb[:], sp[:], ACTF.Exp, scale=1.0 / float(D))
                        emsb = b_sb.tile([128, 128], BF16, tag="emsb")
                        nc.vector.tensor_tensor(emsb[:], esb[:], t_mask[:], op=AL.mult)

                        vp = b_ps.tile([128, 128], F32, tag="vps")
                        for g in range(4):
                            nc.tensor.matmul(
                                vp[32 * g:32 * g + 32, :],
                                t_vti[:, blk * 128 + 32 * g: blk * 128 + 32 * g + 32],
                                t_id16[:], start=True, stop=True,
                                tile_position=(0, 32 * g))
                        vb = b_sb.tile([128, 129], BF16, tag="vb")
                        nc.scalar.activation(vb[:, 0:128], vp[:], ACTF.Copy)
                        nc.gpsimd.memset(vb[:, 128:129], 1.0)

                        cp = b_ps.tile([128, 129], F32, tag="cps")
                        nc.tensor.matmul(cp[:], emsb[:], vb[:], start=True, stop=True)
                        rec = b_sb.tile([128, 1], F32, tag="rec")
                        nc.vector.reciprocal(rec[:], cp[:, 128:129])
                        ctxn = b_sb.tile([128, 128], F32, tag="ctxn")
                        nc.vector.tensor_scalar_mul(ctxn[:], cp[:, 0:128], rec[:])
                        ctp = b_ps.tile([128, 128], F32, tag="ctps")
                        nc.tensor.transpose(ctp[:], ctxn[:], t_idf[:])
                        nc.scalar.activation(
                            t_ctxT[:].rearrange("d (i s) -> d i s", i=H)[:, :,
                                                                        blk * 8:(blk + 1) * 8],
                            ctp[:].rearrange("a (i p) -> a i p", i=H), ACTF.Copy)

                # ---------------- Phase C: dense + residual ----------------
                with (
                    tc.tile_pool(name="c_w", bufs=18) as c_w,
                    tc.tile_pool(name="c_ps", bufs=1, space="PSUM") as c_ps,
                    tc.tile_pool(name="c_sb", bufs=3) as c_sb,
                ):
                    for ow in range(4):
                        psums = [c_ps.tile([128, pos], F32, tag=f"cps{oc}", name=f"cps{ow}_{oc}") for oc in range(4)]
                        for half in range(2):
                            wt = []
                            for cc in range(half * 8, half * 8 + 8):
                                w = c_w.tile([128, 512], wdt, tag="cw")
                                nc.sync.dma_start(w[:], densew[cc * 4 + ow])
                                wt.append(w)
                            for oc in range(4):
                                for ci, cc in enumerate(range(half * 8, half * 8 + 8)):
                                    nc.tensor.matmul(
                                        psums[oc][:],
                                        wt[ci][:, oc * 128:(oc + 1) * 128],
                                        t_ctxT[:, cc * pos:(cc + 1) * pos],
                                        start=(half == 0 and ci == 0),
                                        stop=(half == 1 and ci == 7))
                        for oc in range(4):
                            ot = ow * 4 + oc
                            rt_t = c_sb.tile([128, pos], F32, tag="res")
                            nc.sync.dma_start(rt_t[:], resT[ot])
                            osb = c_sb.tile([128, pos], F32, tag="osb")
                            nc.vector.scalar_tensor_tensor(
                                osb[:], psums[oc][:], t_dnb[:, ot:ot + 1], rt_t[:],
                                op0=AL.add, op1=AL.add)
                            nc.sync.dma_start(outT[ot], osb[:])
                spb_cm.__exit__(None, None, None)
    nc.compile()
    return nc


_CACHED = {}


def _get_nc(pos=POS, use_cc=True):
    key = (pos, use_cc)
    if key not in _CACHED:
        _CACHED[key] = build_bass(pos=pos, use_cc=use_cc)
    return _CACHED[key]


def make_in_maps(hidden_states, residual, alibi, qkv_w, qkv_b, dense_w, dense_b,
                 pos=POS, n_cores=N_CORES, cores_per_batch=4, wdt_np=np.float32):
    nchunk = HID // 128
    n_rw = 3 * HID // 512
    nblk = pos // 8

    qkv_wT = np.ascontiguousarray(qkv_w.T).astype(np.float32)     # [HID, 3HID]
    qkvw_t = np.ascontiguousarray(
        qkv_wT.reshape(nchunk, 128, n_rw, 512).transpose(0, 2, 1, 3)
    ).reshape(nchunk * n_rw, 128, 512).astype(wdt_np)
    dense_wT = np.ascontiguousarray(dense_w.T).astype(np.float32)  # [HID, HID]
    densew_t = np.ascontiguousarray(
        dense_wT.reshape(nchunk, 128, 4, 512).transpose(0, 2, 1, 3)
    ).reshape(nchunk * 4, 128, 512).astype(wdt_np)
    qkvb = np.ascontiguousarray(qkv_b.reshape(48, 128).T).astype(np.float32)
    dnb = np.ascontiguousarray(dense_b.reshape(16, 128).T).astype(np.float32)
    pp = np.arange(8)
    mask = (pp[None, :, None, None] == pp[None, None, None, :])
    mask = np.broadcast_to(mask, (16, 8, 16, 8)).reshape(128, 128)
    mask16 = np.ascontiguousarray(mask.astype(ml_dtypes.bfloat16))

    in_maps = []
    for c in range(n_cores):
        b = c // cores_per_batch
        sq = c % cores_per_batch
        ssl = slice(sq * pos, (sq + 1) * pos)
        hsT = np.ascontiguousarray(hidden_states[b, ssl, :].T).astype(np.float32)
        hsT3 = np.ascontiguousarray(hsT.reshape(nchunk, 128, pos).transpose(1, 0, 2)
                                    ).reshape(128, nchunk * pos).astype(wdt_np)
        rT = np.ascontiguousarray(residual[b, ssl, :].T).astype(np.float32)
        rT3 = np.ascontiguousarray(rT.reshape(16, 128, pos))
        # albc[d, (blk, j, p)] = sqrt(D) * alibi[b*H + j, 0, sq*pos + blk*8 + p]
        al = np.asarray(alibi)[b * H:(b + 1) * H, 0, ssl]          # [H, pos]
        al_bjp = (np.sqrt(float(D)) * al).reshape(H, nblk, 8).transpose(1, 0, 2)
        albc_host = np.ascontiguousarray(
            np.broadcast_to(al_bjp.reshape(1, nblk * 128), (128, nblk * 128))
        ).astype(ml_dtypes.bfloat16)
        in_maps.append({
            "hsT3": hsT3,
            "qkvw": qkvw_t,
            "densew": densew_t,
            "resT": rT3,
            "albc": albc_host,
            "maskbd": mask16,
            "qkvb48": qkvb,
            "denseb16": dnb,
        })
    return in_maps


def kernel(hidden_states, residual, alibi, attention_mask, qkv_w, qkv_b,
           dense_w, dense_b):
    hidden_states = np.asarray(hidden_states, dtype=np.float32)
    residual = np.asarray(residual, dtype=np.float32)
    alibi = np.asarray(alibi, dtype=np.float32)
    qkv_w = np.asarray(qkv_w, dtype=np.float32)
    qkv_b = np.asarray(qkv_b, dtype=np.float32)
    dense_w = np.asarray(dense_w, dtype=np.float32)
    dense_b = np.asarray(dense_b, dtype=np.float32)

    nc = _get_nc()
    in_maps = make_in_maps(hidden_states, residual, alibi, qkv_w, qkv_b,
                           dense_w, dense_b)
    res = run_bass_kernel_spmd(nc, in_maps, core_ids=list(range(N_CORES)))
    out = np.empty((B, S, HID), np.float32)
    for c in range(N_CORES):
        b, sq = c // 4, c % 4
        oT = res.results[c]["outT"]          # [16, 128, POS]
        out[b, sq * POS:(sq + 1) * POS, :] = oT.reshape(HID, POS).T
    return out

